# revision 66
# baseline (speedup 1.0000x reference)
"""Graph-transformer layer (masked dense attention + FFN) on 8 trn2 cores.

Sharding: core c handles batch b = c//2 and query rows
[(c%2)*2048, (c%2)*2048+2048) of that batch; K/V and weights replicated
within the 2-core batch group. The host ships x^T (bf16, per batch) and
adj^T (uint8, per core) during sharding so the device never transposes
activations or the mask, and never does a 4x-cost fp32 matmul: S^T/FFN
run in fp32r (full PE rate at >=256 moving columns), projections and AV
in bf16. Walrus requires fp32/fp32r matmul operands to match dtypes, so
operand pairs are kept same-typed throughout.

Per-core pipeline:
  phase A: K^T[h,k] and Q^T[h,q] = W^T x^T directly from x^T chunks,
           kT fed by the SP DMA queue and qT by the ACT DMA queue
           (interleaved so PE drains both queues at startup); V'[k,h+1]
           = [x Wv | 1] -- the ones column makes AV emit softmax row
           sums for free. Rank-1 ones-row matmuls fold biases when any
           bias is nonzero (separate cached program variant).
  phase B: per 512-query chunk, streamed over 128-key-row pairs:
             S^T psum = K^T.T Q^T          (PE, fp32r, 2x512-col tiles)
             E = exp(S^T/16)               (ACT, psum->sbuf bf16)
             P^T = E * adjT                (Pool, bf16 * uint8)
             AV[q,0:257] += P^T.T V'       (PE, bf16, AV lags S^T by 3
                                            pairs so PE never waits)
           tail: rl = 1/AV[:,256] (DVE), O = AV*rl (DVE/ACT split),
           O^T via PE transpose into the just-freed AV psum banks,
           FF1^T = relu(W1^T O^T), Y = FF1 W2 -> DMA out. Tail PE work
           is deferred into the next chunk's instruction stream so PE
           never drains; the last chunk interleaves each subtile's
           drain with the remaining AV accumulation.
  Softmax skips max-subtraction (scores/16 is O(3), exp can't overflow)
  and uses exp(s)*adj == exp(s + mask) exactly since adj is 0/1.

Cost-model makespan 145.4us/core vs 768.0us for the fp32 baseline.
"""

from contextlib import ExitStack

import numpy as np

B, N, D, H = 4, 4096, 256, 256
NQ = N // 2  # query rows per core
P = 128
QC = 512  # query chunk
NCORES = 8

_CACHE = {}


def _build(with_bias):
    import concourse.bass as bass
    import concourse.bacc as bacc
    import concourse.mybir as mybir
    from concourse.tile import TileContext

    f32 = mybir.dt.float32
    f32r = mybir.dt.float32r
    bf16 = mybir.dt.bfloat16
    u8 = mybir.dt.uint8
    AF = mybir.ActivationFunctionType

    n_qc = NQ // QC  # 4 query chunks
    n_kb = N // P  # 32 key blocks
    n_kbp = n_kb // 2  # 16 key block pairs
    DT = D // P  # 2
    HT = H // P  # 2

    nc = bacc.Bacc("TRN2", target_bir_lowering=False)

    xT_d = nc.dram_tensor("xT", [D, N], bf16, kind="ExternalInput").ap()
    xTq_d = nc.dram_tensor("xTq", [D, NQ], bf16, kind="ExternalInput").ap()
    adjT_d = nc.dram_tensor("adjT", [N, NQ], u8, kind="ExternalInput").ap()
    w_d = {}
    for nm in ("Wq", "Wk", "Wv"):
        w_d[nm] = nc.dram_tensor(nm, [256, 256], bf16, kind="ExternalInput").ap()
    for nm in ("W1", "W2"):
        w_d[nm] = nc.dram_tensor(nm, [256, 256], f32r, kind="ExternalInput").ap()
    if with_bias:
        br_d = {
            nm: nc.dram_tensor(nm, [1, 256], f32r, kind="ExternalInput").ap()
            for nm in ("bq", "bk", "bv", "b2")
        }
        b1c_d = nc.dram_tensor("b1c", [256, 1], f32r, kind="ExternalInput").ap()
        ones_d = nc.dram_tensor("ones_in", [1, QC], f32r, kind="ExternalInput").ap()
    ident_d = nc.dram_tensor("ident_in", [P, P], f32r, kind="ExternalInput").ap()
    out_d = nc.dram_tensor("out", [NQ, D], bf16, kind="ExternalOutput").ap()

    with ExitStack() as ctx:
        tc = ctx.enter_context(TileContext(nc))
        const = ctx.enter_context(tc.tile_pool(name="const", bufs=1))
        xT_p = ctx.enter_context(tc.tile_pool(name="xT", bufs=1))
        kT_p = ctx.enter_context(tc.tile_pool(name="kT", bufs=1))
        qT_p = ctx.enter_context(tc.tile_pool(name="qT", bufs=1))
        v_p = ctx.enter_context(tc.tile_pool(name="v", bufs=1))
        adj_p = ctx.enter_context(tc.tile_pool(name="adj", bufs=12))
        e_p = ctx.enter_context(tc.tile_pool(name="e", bufs=5))
        pm_p = ctx.enter_context(tc.tile_pool(name="pm", bufs=5))
        rl_p = ctx.enter_context(tc.tile_pool(name="rl", bufs=4))
        on_p = ctx.enter_context(tc.tile_pool(name="on", bufs=4))
        ot_p = ctx.enter_context(tc.tile_pool(name="ot", bufs=2))
        ff_p = ctx.enter_context(tc.tile_pool(name="ff", bufs=2))
        y_p = ctx.enter_context(tc.tile_pool(name="y", bufs=3))
        st_ps = ctx.enter_context(tc.tile_pool(name="st_ps", bufs=2, space="PSUM"))
        av_ps = ctx.enter_context(tc.tile_pool(name="av_ps", bufs=1, space="PSUM"))

        # ---- constants; DMA order matters: first kT matmul needs Wk (+ bk,
        # ones when with_bias) and xT chunk 0 only ----
        w_sb = {nm: const.tile([P, DT, 256],
                               bf16 if nm in ("Wk", "Wq", "Wv") else f32r,
                               tag=f"w_{nm}", name=f"w_{nm}")
                for nm in ("Wk", "Wq", "Wv", "W1", "W2")}
        if with_bias:
            br_sb = {nm: const.tile([1, 256], f32r, tag=f"b_{nm}", name=f"b_{nm}")
                     for nm in ("bk", "bq", "bv", "b2")}
            ones = const.tile([1, QC], f32r, tag="ones")
            b1c = const.tile([P, HT], f32r, tag="b1c")
        ident = const.tile([P, P], f32r)
        xT = xT_p.tile([P, DT, N], bf16)  # x^T [d%128, d//128, n]
        xTq = xT_p.tile([P, DT, NQ], bf16, tag="xTq")
        kT = kT_p.tile([P, HT, N], f32r)  # K^T [h%128, h//128, k]
        qT = qT_p.tile([P, HT, NQ], f32r)  # Q^T [h%128, h//128, q]
        v_sb = v_p.tile([P, n_kb, H + 1], bf16)  # V' [k%128, k//128, h|1]

        def dma_w(nm, eng=None):
            (eng or nc.sync).dma_start(
                w_sb[nm][:], w_d[nm][:].rearrange("(i p) c -> p i c", i=DT)
            )

        dma_w("Wk")
        if with_bias:
            nc.sync.dma_start(br_sb["bk"][:], br_d["bk"][:])
            nc.sync.dma_start(ones[:], ones_d[:])
        # x^T DMAs interleaved by dt so early projections start sooner
        for ch in range(N // QC):
            csl = slice(ch * QC, (ch + 1) * QC)
            nc.sync.dma_start(
                xT[:, :, csl], xT_d[:, csl].rearrange("(i p) c -> p i c", i=DT)
            )
        dma_w("Wq", nc.scalar)
        if with_bias:
            nc.sync.dma_start(br_sb["bq"][:], br_d["bq"][:])
        for ch in range(NQ // QC):
            csl = slice(ch * QC, (ch + 1) * QC)
            nc.scalar.dma_start(
                xTq[:, :, csl], xTq_d[:, csl].rearrange("(i p) c -> p i c", i=DT)
            )
        dma_w("Wv")
        if with_bias:
            nc.sync.dma_start(br_sb["bv"][:], br_d["bv"][:])
            nc.sync.dma_start(br_sb["b2"][:], br_d["b2"][:])
            for ht in range(HT):
                nc.sync.dma_start(b1c[:, ht : ht + 1], b1c_d[ht * P : (ht + 1) * P, :])
        dma_w("W1", nc.scalar)
        dma_w("W2", nc.scalar)
        nc.scalar.dma_start(ident[:], ident_d[:])
        nc.vector.memset(v_sb[:, :, H : H + 1], 1.0)

        # ---- phase A: projections ----
        pa_cnt = [0]

        def pa_psum(width):
            """Rotate phase-A psum over st pool AND the idle av banks."""
            i = pa_cnt[0]
            pa_cnt[0] += 1
            if i % 2 == 0:
                ps = st_ps.tile([P, 2, QC], f32, tag="st", name="ps")
                return ps[:, 0, 0:width]
            ps = av_ps.tile([P, QC], f32, tag=f"av{(i // 2) % 4}", name="ps")
            return ps[:, 0:width]

        def proj_unit(dst, src, ht, ch, wname, bname, on_dve):
            """dst[:, ht, ch-chunk] = (W^T x + b)^T for one 512 chunk."""
            csl = slice(ch * QC, (ch + 1) * QC)
            ps = pa_psum(QC)
            for dt in range(DT):
                nc.tensor.matmul(
                    ps,
                    w_sb[wname][:, dt, ht * P : (ht + 1) * P],
                    src[:, dt, csl],
                    start=(dt == 0),
                    stop=(not with_bias and dt == DT - 1),
                )
            if with_bias:
                nc.tensor.matmul(
                    ps,
                    br_sb[bname][0:1, ht * P : (ht + 1) * P],
                    ones[0:1, :],
                    start=False,
                    stop=True,
                )
            if on_dve:  # split copy load DVE/ACT
                nc.vector.tensor_copy(dst[:, ht, csl], ps)
            else:
                nc.scalar.copy(dst[:, ht, csl], ps)

        # kT (fed by SP xT stream) interleaved with qT (fed by ACT xTq
        # stream) so PE drains two DMA queues at once at startup
        units = []
        for ch in range(N // QC):
            for ht in range(HT):
                units.append((kT, xT, ht, ch, "Wk", "bk"))
        qunits = []
        for ch in range(NQ // QC):
            for ht in range(HT):
                qunits.append((qT, xTq, ht, ch, "Wq", "bq"))
        mixed = []
        qi = 0
        for i, u in enumerate(units):
            mixed.append(u)
            if i % 2 == 1 and qi < len(qunits):
                mixed.append(qunits[qi])
                qi += 1
        mixed.extend(qunits[qi:])
        for i, (dst, src, ht, ch, wn, bn) in enumerate(mixed):
            proj_unit(dst, src, ht, ch, wn, bn, i % 2 == 0)
        for kbp in range(n_kb // 2):  # V two key blocks per psum tile
            ps = pa_psum(2 * H)
            for j in range(2):
                kb = 2 * kbp + j
                sub = ps.rearrange("p (j c) -> p j c", j=2)[:, j, :]
                for dt in range(DT):
                    nc.tensor.matmul(
                        sub,
                        xT[:, dt, kb * P : (kb + 1) * P],
                        w_sb["Wv"][:, dt, :],
                        start=(dt == 0),
                        stop=(not with_bias and dt == DT - 1),
                    )
                if with_bias:
                    nc.tensor.matmul(
                        sub, ones[0:1, 0:P], br_sb["bv"][0:1, :],
                        start=False, stop=True,
                    )
            dst = v_sb[:, 2 * kbp : 2 * kbp + 2, 0:H]
            view = ps.rearrange("p (j c) -> p j c", j=2)
            if kbp % 2 == 0:
                nc.vector.tensor_copy(dst, view)
            else:
                nc.scalar.copy(dst, view)

        # ---- phase B ----
        inv_s = float(1.0 / np.sqrt(np.float32(H)))

        def make_tail(av, qc):
            """Per-chunk tail emitters; each method is one PE-light piece."""
            qbase = qc * QC
            last = qc == n_qc - 1
            oNs = [None] * 4

            def drain(qs, on_dve):
                rl = rl_p.tile([P, 1], f32, name="rl")
                nc.vector.reciprocal(rl[:], av[qs][:, H : H + 1])
                oN = on_p.tile([P, H], f32r, name="oN")
                if on_dve:
                    nc.vector.tensor_scalar(
                        oN[:], av[qs][:, 0:H], rl[:], None, mybir.AluOpType.mult
                    )
                else:
                    nc.scalar.mul(oN[:], av[qs][:, 0:H], rl[:])
                oNs[qs] = oN

            oT = ot_p.tile([P, HT, QC], f32r, name="oT")
            ff = ff_p.tile([P, HT, QC], f32r, name="ff")
            y_sb = y_p.tile([P, 4, H], bf16, name="y")

            def piece_tp(qs):
                # transposes borrow av[qs]'s bank: free right after oN reads
                tp = av_ps.tile([P, QC], f32r, tag=f"av{qs}", name="tp")
                for hh in range(HT):
                    nc.tensor.matmul(
                        tp[:, hh * P : (hh + 1) * P],
                        oNs[qs][:, hh * P : (hh + 1) * P],
                        ident[:],
                        is_transpose=True,
                        start=True,
                        stop=True,
                    )
                # one strided copy moves both h-halves to oT
                dst = oT[:, :, qs * P : (qs + 1) * P]
                view = tp[0:P, 0:2 * P].rearrange("p (hh c) -> p hh c", hh=2)
                if last and qs % 2 == 1:
                    nc.scalar.copy(dst, view)
                else:
                    nc.vector.tensor_copy(dst, view)

            def piece_ff(h2, strip=None):
                # own tile per call: avoids tile-granular WAR with the relu
                ssl = slice(0, QC) if strip is None else slice(
                    strip * (QC // 2), (strip + 1) * (QC // 2))
                ff_ps = st_ps.tile([P, 2, QC], f32, tag="st", name="ffps")
                out_ps = ff_ps[:, 0, 0 : ssl.stop - ssl.start]
                for hh in range(HT):
                    nc.tensor.matmul(
                        out_ps,
                        w_sb["W1"][:, hh, h2 * P : (h2 + 1) * P],
                        oT[:, hh, ssl],
                        start=(hh == 0),
                        stop=(hh == HT - 1),
                    )
                if with_bias:
                    nc.scalar.activation(
                        ff[:, h2, ssl], out_ps, AF.Relu,
                        bias=b1c[:, h2 : h2 + 1],
                    )
                elif h2 == 0:
                    nc.scalar.activation(ff[:, h2, ssl], out_ps, AF.Relu)
                else:
                    nc.vector.tensor_relu(ff[:, h2, ssl], out_ps)

            def piece_y(qs):
                y_ps = st_ps.tile([P, 2, QC], f32, tag="st", name="yps")
                yp = y_ps[:, 0, 0:H]
                for h2 in range(HT):
                    nc.tensor.matmul(
                        yp,
                        ff[:, h2, qs * P : (qs + 1) * P],
                        w_sb["W2"][:, h2, :],
                        start=(h2 == 0),
                        stop=(not with_bias and h2 == HT - 1),
                    )
                if with_bias:
                    nc.tensor.matmul(
                        yp, ones[0:1, 0:P], br_sb["b2"][0:1, :],
                        start=False, stop=True,
                    )
                if last and qs % 2 == 1:  # fan the drain over idle engines
                    nc.scalar.copy(y_sb[:, qs, :], yp)
                    nc.scalar.dma_start(
                        out_d[qbase + qs * P : qbase + (qs + 1) * P, :],
                        y_sb[:, qs, :],
                    )
                else:
                    nc.vector.tensor_copy(y_sb[:, qs, :], yp)
                    nc.sync.dma_start(
                        out_d[qbase + qs * P : qbase + (qs + 1) * P, :],
                        y_sb[:, qs, :],
                    )

            import types
            return types.SimpleNamespace(
                drain=drain, tp=piece_tp, ff=piece_ff, y=piece_y
            )

        pending = []
        for qc in range(n_qc):
            qsl = slice(qc * QC, (qc + 1) * QC)
            av = None
            LAG = 3  # AV trails S^T by 2 pairs so PE never waits on ACT/Pool
            pms = {}
            for kbp in range(n_kbp + LAG):
                if pending and kbp == 1:
                    for _ in range(4):  # all tp pieces before av realloc
                        pending.pop(0)()
                if kbp == LAG:
                    av = []
                    for i in range(4):
                        av_t = av_ps.tile([P, QC], f32, tag=f"av{i}", name="av_t")
                        av.append(av_t)
                if kbp < n_kbp:
                    st = st_ps.tile([P, 2, QC], f32, tag="st", name="st")
                    adj_t = adj_p.tile([P, 2, QC], u8, name="adj_t")
                    nc.sync.dma_start(
                        adj_t[:],
                        adjT_d[2 * kbp * P : (2 * kbp + 2) * P, qsl].rearrange(
                            "(j p) c -> p j c", j=2
                        ),
                    )
                    for j in range(2):
                        ksl = slice((2 * kbp + j) * P, (2 * kbp + j + 1) * P)
                        for ht in range(HT):
                            nc.tensor.matmul(
                                st[:, j, :],
                                kT[:, ht, ksl],
                                qT[:, ht, qsl],
                                start=(ht == 0),
                                stop=(ht == HT - 1),
                            )
                    e_t = e_p.tile([P, 2, QC], bf16, name="e_t")
                    nc.scalar.activation(e_t[:], st[:], AF.Exp, scale=inv_s)
                    pm = pm_p.tile([P, 2, QC], bf16, name="pm")
                    nc.gpsimd.tensor_mul(pm[:], e_t[:], adj_t[:])
                    pms[kbp] = pm
                if kbp >= LAG:
                    pm_av = pms.pop(kbp - LAG)
                    final_last = (qc == n_qc - 1) and kbp == n_kbp + LAG - 1
                    if final_last:
                        tail = make_tail(av, qc)
                    for j in range(2):
                        kb = 2 * (kbp - LAG) + j
                        for qs in range(4):
                            nc.tensor.matmul(
                                av[qs][:, 0 : H + 1],
                                pm_av[:, j, qs * P : (qs + 1) * P],
                                v_sb[:, kb, :],
                                start=(kb == 0),
                                stop=(kb == n_kb - 1),
                            )
                            if final_last and j == 1:
                                # av[qs] closed: start its drain while the
                                # remaining qs still accumulate
                                tail.drain(qs, qs % 2 == 0)
                                if qs >= 1:
                                    tail.tp(qs - 1)
                    if final_last:
                        # half-width FFN strips shorten the relu->Y chain
                        tail.tp(3)
                        tail.ff(0, 0)
                        tail.ff(1, 0)
                        tail.y(0)
                        tail.ff(0, 1)
                        tail.y(1)
                        tail.ff(1, 1)
                        tail.y(2)
                        tail.y(3)
                if pending and kbp >= LAG:
                    pending.pop(0)()
            if qc < n_qc - 1:
                while pending:  # safety: never carry more than one chunk back
                    pending.pop(0)()
                tail = make_tail(av, qc)
                tail.drain(0, True)
                tail.drain(3, False)
                tail.drain(1, True)
                tail.drain(2, False)
                pending = [lambda t=tail: t.tp(0), lambda t=tail: t.tp(1),
                           lambda t=tail: t.tp(2), lambda t=tail: t.tp(3),
                           lambda t=tail: t.ff(0), lambda t=tail: t.ff(1),
                           lambda t=tail: t.y(0), lambda t=tail: t.y(1),
                           lambda t=tail: t.y(2), lambda t=tail: t.y(3)]

    return nc


def _get_nc(with_bias):
    key = f"nc{int(with_bias)}"
    if key not in _CACHE:
        nc = _build(with_bias)
        nc.finalize()
        _CACHE[key] = nc
    return _CACHE[key]


def kernel(x, adj, Wq, bq, Wk, bk, Wv, bv, W1, b1, W2, b2):
    from concourse.bass_utils import run_bass_kernel_spmd

    x = np.ascontiguousarray(np.asarray(x, dtype=np.float32))
    adj = np.asarray(adj)
    biases = [np.asarray(b, np.float32) for b in (bq, bk, bv, b1, b2)]
    with_bias = any(np.any(b) for b in biases)
    import ml_dtypes
    weights = {
        "Wq": np.ascontiguousarray(np.asarray(Wq, np.float32).astype(ml_dtypes.bfloat16)),
        "Wk": np.ascontiguousarray(np.asarray(Wk, np.float32).astype(ml_dtypes.bfloat16)),
        "Wv": np.ascontiguousarray(np.asarray(Wv, np.float32).astype(ml_dtypes.bfloat16)),
        "W1": np.ascontiguousarray(np.asarray(W1, np.float32)),
        "W2": np.ascontiguousarray(np.asarray(W2, np.float32)),
    }
    if with_bias:
        weights.update({
            "bq": np.ascontiguousarray(biases[0].reshape(1, 256)),
            "bk": np.ascontiguousarray(biases[1].reshape(1, 256)),
            "bv": np.ascontiguousarray(biases[2].reshape(1, 256)),
            "b1c": np.ascontiguousarray(biases[3].reshape(256, 1)),
            "b2": np.ascontiguousarray(biases[4].reshape(1, 256)),
        })
    nc = _get_nc(with_bias)
    in_maps = []
    for b in range(B):
        xT = np.ascontiguousarray(x[b].T.astype(ml_dtypes.bfloat16))  # [D, N]
        adjT = np.asarray(adj[b], np.uint8).T  # [N, N] view
        for half in range(2):
            q0 = half * NQ
            m = {
                "xT": xT,
                "xTq": np.ascontiguousarray(xT[:, q0 : q0 + NQ]),
                "adjT": np.ascontiguousarray(adjT[:, q0 : q0 + NQ]),
            }
            m.update(weights)
            m["ident_in"] = np.eye(P, dtype=np.float32)
            if with_bias:
                m["ones_in"] = np.ones((1, QC), dtype=np.float32)
            in_maps.append(m)
    global _last_in_maps
    _last_in_maps = in_maps
    res = run_bass_kernel_spmd(nc, in_maps, list(range(NCORES)))
    out = np.empty((B, N, D), dtype=np.float32)
    for c in range(NCORES):
        b, half = c // 2, c % 2
        q0 = half * NQ
        out[b, q0 : q0 + NQ] = res.results[c]["out"]
    return out
